# revision 22
# baseline (speedup 1.0000x reference)
"""Trainium2 Bass kernel for nn_BrainBottleneckLocal (dense_cnn).

Sharding: spatial rows. H=16 rows are split 2-per-core across 8 NeuronCores;
every layer is then core-local (the LC weight is per-location, so the 604 MB
lc_w tensor splits 8x by row — the dominant DMA stream).

Single-shot latency is DMA-bound: the per-core DMA budget is ~33 MB
(lcw fp8 18.9 + x bf16 8.4 + consts 1.2 + out bf16 4.2) against one
~360 GB/s DMA-engine pool => ~91 us floor. Per-queue DMA issue latency is
~2.7 us, so the stream is packed into FEW HUGE DMAs and spread over the
three issue queues:
  - SP: w1 weights, then the lcw stream as 4 chunks (4.7 MB each), then the
    4 output slice-stores (traced later => behind lcw in the FIFO, so they
    drain during the compute tail without delaying the weight stream).
  - Act: x in 2 DMAs (rows 0-2: 6.3 MB, row 3: 2.1 MB).
  - Pool/SWDGE: small consts (cbf/cf4/ga), done in the first ~10 us.
The trace interleaves each LC chunk with its dependent conv3/inhibition
512-slice so the PE FIFO pipelines the tail (chunk k's slice runs while
chunk k+1 streams in).

Per-core pipeline (free-dim layout is (h, w, n) everywhere):
  1. conv1x1 #1 + BN1 + ReLU on the core's 2 rows plus a 1-row halo each side
     (4 rows, boundary rows zero-padded by the host). Output is written
     straight to fp8-e4m3 (the LC input quantization).
  2. locally-connected 3x3 + BN2 + ReLU: fp8 weights (per-out-channel pow2
     scale folded in, undone by BN2's per-partition activation scale) and fp8
     patches via the tensor engine's DoubleRow perf mode (2 fp8 MACs/cell).
  3. conv1x1 #2 + BN3, residual add (PSUM-accumulated identity matmul vs the
     bf16 conv1 input tile), ReLU -> resb bf16.
  4. opponent inhibition through a low-rank factorization of the mixing
     matrix g (host-side SVD; sigma ~ C/8 makes g numerically rank <~16):
     inh = A @ (B @ resb), then out = resb / (1 + inh), stored bf16.
Matmuls accumulate in fp32 PSUM. BN scales are folded into weights on the
host; BN biases apply via per-partition activation bias. All cores run an
identical program; only per-core data differs (boundary handling = zeroed
LC taps).
"""

from contextlib import ExitStack

import numpy as np

import concourse.bacc as bacc
import concourse.mybir as mybir
import concourse.tile as tile
from concourse.bass_utils import run_bass_kernel_spmd

F32 = mybir.dt.float32
BF16 = mybir.dt.bfloat16
FP8 = mybir.dt.float8e4
NPBF16 = mybir.dt.np(BF16)
NPFP8 = mybir.dt.np(FP8)

EPS = 1e-5
N, CIN, H, W = 64, 1024, 16, 16
WID, COUT = 256, 1024
NCORES = 8
RPC = H // NCORES          # rows per core = 2
HLO = RPC + 2              # rows incl halo = 4
WP = W + 2                 # padded width = 18
NLOC = RPC * W             # LC locations per core = 32
CC1 = CIN // 128           # 8
CCW = WID // 128           # 2
CC3 = COUT // 128          # 8
FR = RPC * W * N           # free size of per-core row block = 2048
RANK = 16                  # low-rank size for the inhibition mixing matrix
NCHUNK = 8                 # lcw stream chunks (4 locations each)
LPC = NLOC // NCHUNK       # locations per chunk = 8
XBF = HLO * W * N          # per-conv1-chunk free size = 4096
# packed bf16 consts width: w3t | gbt | 128x128 identity | 64x64 identity
CBF_X = CCW * COUT + CC3 * RANK + 128 + 64
AF = mybir.ActivationFunctionType
ALU = mybir.AluOpType
DR = mybir.MatmulPerfMode.DoubleRow


def _declare_drams(nc, variant):
    ap = {}
    # x, (p, (cc, h, w, n)) so conv1 chunks are column ranges of one tile
    ap["xb"] = nc.dram_tensor("xb", [128, CC1 * XBF], BF16,
                              kind="ExternalInput").ap()
    # lcw stream: 8 chunks x 4 locations, 18.4KB DMA lines
    ap["lcw"] = nc.dram_tensor("lcw", [NCHUNK, 128, LPC * 9 * 2 * WID], FP8,
                               kind="ExternalInput").ap()
    # conv1 weights alone (first thing conv1 needs)
    ap["w1c"] = nc.dram_tensor("w1c", [128, CC1 * WID], BF16,
                               kind="ExternalInput").ap()
    # packed constants:
    #   cbf: [w3t (2*1024) | gbt (8*16) | I128 | I64] bf16, 128-part
    #   cf4: [b1 (2) | b2 (2) | s2 (2) | b3 (8)] f32 columns, 128-part
    ap["cbf"] = nc.dram_tensor("cbf", [128, CBF_X], BF16,
                               kind="ExternalInput").ap()
    ap["cf4"] = nc.dram_tensor("cf4", [128, 14], F32,
                               kind="ExternalInput").ap()
    if variant == "lr":
        # row RANK of ga is all-ones: stage-2 matmul then yields 1 + inh
        ap["ga"] = nc.dram_tensor("ga", [RANK + 1, COUT], BF16,
                                  kind="ExternalInput").ap()
    else:
        ap["gd"] = nc.dram_tensor("gd", [CC3, 128, COUT], BF16,
                                  kind="ExternalInput").ap()
    # out, (p, (oc, hl, j, n)) so a 512-slice store spans all 8 oc chunks
    ap["out"] = nc.dram_tensor("out", [128, CC3 * FR], BF16,
                               kind="ExternalOutput").ap()
    return ap


ALL_STAGES = ("conv1", "lcdma", "lcmm", "conv3", "inhib")


def _build_nc(ktimes: int = 1, variant: str = "lr", stages=ALL_STAGES):
    nc = bacc.Bacc("TRN2", target_bir_lowering=False, debug=False,
                   num_devices=NCORES)
    ap = _declare_drams(nc, variant)
    with tile.TileContext(nc) as tc:
        if ktimes == 1:
            _trace_kernel(tc, nc, ap, variant, stages)
        else:
            with tc.For_i(0, ktimes, 1):
                _trace_kernel(tc, nc, ap, variant, stages)
    nc.compile()
    return nc


def _trace_kernel(tc, nc, ap, variant="lr", stages=ALL_STAGES):
    with ExitStack() as ctx:
        persist = ctx.enter_context(tc.tile_pool(name="persist", bufs=1))
        psum = ctx.enter_context(
            tc.tile_pool(name="psum", bufs=3, space="PSUM"))

        # ---- SP queue: x rows 0-1, even lcw chunks, stores ---------------
        # (w1c goes via SWDGE so x leads the HWDGE queues)
        w1c_t = persist.tile([128, CC1 * WID], BF16, name="w1c", tag="w1c")
        nc.gpsimd.dma_start(out=w1c_t, in_=ap["w1c"])
        xball = persist.tile([128, CC1 * XBF], BF16, name="xball", tag="xb")
        xv = xball.rearrange("p (c f) -> p c f", c=CC1)
        xdv = ap["xb"].rearrange("p (c f) -> p c f", c=CC1)
        nc.sync.dma_start(out=xv[:, :, :W * N], in_=xdv[:, :, :W * N])
        nc.sync.dma_start(out=xv[:, :, W * N:2 * W * N],
                          in_=xdv[:, :, W * N:2 * W * N])
        # ---- Act queue: x row 2 first, then odd lcw chunks, then row 3 ----
        nc.scalar.dma_start(out=xv[:, :, 2 * W * N:3 * W * N],
                            in_=xdv[:, :, 2 * W * N:3 * W * N])
        lcw_pool = ctx.enter_context(tc.tile_pool(name="lcwp", bufs=4))
        lw_tiles = []
        if "lcdma" in stages:
            for ck in range(NCHUNK):
                lw = lcw_pool.tile([128, LPC * 9 * 2 * WID], FP8,
                                   name="lcw_t", tag="lcw")
                eng = (nc.sync, nc.scalar)[ck % 2]
                eng.dma_start(out=lw, in_=ap["lcw"][ck])
                lw_tiles.append(lw)
        # x row 3 behind the odd chunks: conv1 h=3 is traced after slice 1,
        # by which time this has landed
        nc.scalar.dma_start(out=xv[:, :, 3 * W * N:],
                            in_=xdv[:, :, 3 * W * N:])

        # ---- Pool/SWDGE queue: small consts -------------------------------
        cbf_t = persist.tile([128, CBF_X], BF16, name="cbf", tag="cbf")
        nc.gpsimd.dma_start(out=cbf_t, in_=ap["cbf"])
        cf4_t = persist.tile([128, 14], F32, name="cf4", tag="cf4")
        nc.gpsimd.dma_start(out=cf4_t, in_=ap["cf4"])

        w1t_t = [w1c_t[:, cc * WID:(cc + 1) * WID] for cc in range(CC1)]
        off = 0
        w3t_t = [cbf_t[:, off + oc * COUT:off + (oc + 1) * COUT]
                 for oc in range(CCW)]
        off += CCW * COUT
        gbt_t = [cbf_t[:, off + cc * RANK:off + (cc + 1) * RANK]
                 for cc in range(CC3)]
        off += CC3 * RANK
        i128_t = cbf_t[:, off:off + 128]
        ident_t = cbf_t[0:64, off + 128:off + 128 + 64]
        b1_t = [cf4_t[:, c:c + 1] for c in range(CCW)]
        b2_t = [cf4_t[:, 2 + c:3 + c] for c in range(CCW)]
        s2_t = [cf4_t[:, 4 + c:5 + c] for c in range(CCW)]
        b3_t = [cf4_t[:, 6 + c:7 + c] for c in range(CC3)]
        if variant == "lr":
            ga_t = persist.tile([RANK + 1, COUT], BF16, name="ga", tag="ga")
            nc.gpsimd.dma_start(out=ga_t, in_=ap["ga"])
        else:
            gd_t = []
            for cc in range(CC3):
                t = persist.tile([128, COUT], BF16, name=f"gd_{cc}",
                                 tag=f"gd{cc}")
                nc.gpsimd.dma_start(out=t, in_=ap["gd"][cc])
                gd_t.append(t)

        out2_t = [persist.tile([128, FR], BF16, name=f"out2_{oc}",
                               tag=f"out2{oc}") for oc in range(CCW)]
        # single resb tile so one DMA stores a 512-slice across all 8 oc
        resb = persist.tile([128, CC3 * FR], BF16, name="resb", tag="resb")
        rv = resb.rearrange("p (c f) -> p c f", c=CC3)
        odv = ap["out"].rearrange("p (c f) -> p c f", c=CC3)

        # out1 fp8, padded width: [128, (c2, h4, w18, n64)], pad cols zeroed
        out1q = persist.tile([128, CCW * HLO * WP * N], FP8, name="out1q",
                             tag="out1q")
        o1v = out1q.rearrange("p (c h w n) -> p c h w n",
                              c=CCW, h=HLO, w=WP)
        nc.gpsimd.memset(o1v[:, :, :, 0, :], 0.0)
        nc.gpsimd.memset(o1v[:, :, :, W + 1, :], 0.0)

        lct_pool = ctx.enter_context(tc.tile_pool(name="lctp", bufs=2))
        div_pool = ctx.enter_context(tc.tile_pool(name="divp", bufs=4))
        yb_t = None
        if variant == "lr":
            # moving operand of inhibition stage 2; row RANK stays 1.0.
            yb_t = [persist.tile([RANK + 1, 512], BF16, name=f"yb{i}",
                                 tag=f"yb{i}") for i in range(2)]
            for t in yb_t:
                nc.gpsimd.memset(t, 1.0)

        def conv1_rows(rows):
            # conv1x1 #1 + BN1 + ReLU -> padded fp8 out1 (skip W-pad cols)
            for h in rows:
                for oc in range(CCW):
                    for q in range(2):
                        ps = psum.tile([128, 512], F32, name="ps1", tag="a",
                                       bufs=2)
                        base = h * (W * N) + q * 512
                        for cc in range(CC1):
                            nc.tensor.matmul(
                                ps,
                                w1t_t[cc][:, oc * 128:(oc + 1) * 128],
                                xball[:,
                                      cc * XBF + base:cc * XBF + base + 512],
                                start=(cc == 0), stop=(cc == CC1 - 1))
                        nc.scalar.activation(
                            out=o1v[:, oc, h, 1 + 8 * q:9 + 8 * q, :],
                            in_=ps.rearrange("p (w n) -> p w n", n=N),
                            func=AF.Relu, bias=b1_t[oc], scale=1.0)

        if "conv1" in stages:
            conv1_rows(range(3))   # rows 0-2; row 3 traced after slice 1

        if "lcmm" not in stages:
            for oc in range(CCW):
                nc.gpsimd.memset(out2_t[oc], 0.01)
        lw_shared = None
        if "lcdma" not in stages and "lcmm" in stages:
            lw_shared = persist.tile([128, LPC * 9 * 2 * WID], FP8,
                                     name="lw_shared", tag="lws")
            nc.gpsimd.memset(lw_shared, 0.01)
        if "conv3" not in stages:
            nc.gpsimd.memset(resb, 0.01)

        for ck in range(NCHUNK):
            # -- LC chunk: 4 locations -> out2 cols [ck*256, ck*256+256) --
            if "lcmm" in stages:
                lw = lw_tiles[ck] if "lcdma" in stages else lw_shared
                lwv = lw.rearrange("p (l dk c o) -> p l dk c o",
                                   l=LPC, dk=9, c=2)
                pst_all = psum.tile([128, CCW * LPC * N], BF16, name="pst",
                                    tag="tp", bufs=1)
                pst = [pst_all[:, oc * LPC * N:(oc + 1) * LPC * N]
                       for oc in range(CCW)]
                for lp in range(LPC // 2):
                    # two locations accumulate into one PSUM bank so the
                    # psum->sbuf copy is one big transfer, alternating
                    # between DVE and Act so neither paces the LC loop
                    ps2 = psum.tile([64, 2 * WID], F32, name="ps2",
                                    tag="lc", bufs=2)
                    for half in range(2):
                        li = lp * 2 + half
                        loc = ck * LPC + li
                        hl, j = divmod(loc, W)
                        po = half * WID
                        for dk in range(9):
                            di, dj = divmod(dk, 3)
                            nc.tensor.matmul(
                                ps2[:, po:po + WID],
                                o1v[:, :, hl + di, j + dj, :],
                                lwv[:, li, dk],
                                start=(dk == 0), stop=(dk == 8),
                                perf_mode=DR)
                    tmpb = lct_pool.tile([64, 2 * WID], BF16, name="tmpb",
                                         tag="tmpb")
                    if lp % 2 == 0:
                        nc.vector.tensor_copy(out=tmpb, in_=ps2)
                    else:
                        nc.scalar.activation(out=tmpb, in_=ps2,
                                             func=AF.Copy, scale=1.0)
                    for half in range(2):
                        li = lp * 2 + half
                        for oc in range(CCW):
                            hb = half * WID + oc * 128
                            nc.tensor.transpose(
                                pst[oc][:, li * N:(li + 1) * N],
                                tmpb[:, hb:hb + 128], ident_t)
                for oc in range(CCW):
                    nc.scalar.activation(
                        out=out2_t[oc][:, ck * LPC * N:(ck + 1) * LPC * N],
                        in_=pst[oc], func=AF.Relu, bias=b2_t[oc],
                        scale=s2_t[oc])

            if ck % 2 == 0:
                continue
            # -- slice ns = ck//2: conv3 + BN3 + residual + ReLU, inhibition,
            # divide, store --
            ns = ck // 2
            sl = slice(ns * 512, ns * 512 + 512)
            for oc3 in range(CC3 if "conv3" in stages else 0):
                ps = psum.tile([128, 512], F32, name="ps3", tag="a", bufs=2)
                for oc in range(CCW):
                    nc.tensor.matmul(
                        ps, w3t_t[oc][:, oc3 * 128:(oc3 + 1) * 128],
                        out2_t[oc][:, sl],
                        start=(oc == 0), stop=False)
                # residual add rides the PSUM accumulator: ps += I @ x
                rb = oc3 * XBF + W * N + ns * 512
                nc.tensor.matmul(ps, i128_t, xball[:, rb:rb + 512],
                                 start=False, stop=True)
                # resb = relu(ps + beta3); alternate Act/DVE so the 8
                # BN3 ops do not serialize on one engine ahead of yps
                if oc3 % 2 == 0:
                    nc.scalar.activation(out=rv[:, oc3, sl], in_=ps,
                                         func=AF.Relu, bias=b3_t[oc3],
                                         scale=1.0)
                else:
                    nc.vector.tensor_scalar(
                        out=rv[:, oc3, sl], in0=ps, scalar1=b3_t[oc3],
                        scalar2=0.0, op0=ALU.add, op1=ALU.max)
            if "inhib" in stages:
                if variant == "lr":
                    yps = psum.tile([RANK, 512], F32, name="yps", tag="lc",
                                    bufs=2)
                    for cc in range(CC3):
                        nc.tensor.matmul(yps, gbt_t[cc], rv[:, cc, sl],
                                         start=(cc == 0),
                                         stop=(cc == CC3 - 1))
                    yb = yb_t[ns % 2]
                    nc.vector.tensor_copy(out=yb[:RANK], in_=yps)
                for oc in range(CC3):
                    ps = psum.tile([128, 512], F32, name="ps4", tag="s4",
                                   bufs=3)
                    if variant == "lr":
                        # lhsT row RANK is ones, yb row RANK is ones:
                        # psum = inh + 1 directly
                        nc.tensor.matmul(
                            ps, ga_t[:, oc * 128:(oc + 1) * 128],
                            yb, start=True, stop=True)
                        den = ps
                    else:
                        for cc in range(CC3):
                            nc.tensor.matmul(
                                ps, gd_t[cc][:, oc * 128:(oc + 1) * 128],
                                rv[:, cc, sl],
                                start=(cc == 0), stop=(cc == CC3 - 1))
                        den = div_pool.tile([128, 512], F32, name="den",
                                            tag="den")
                        nc.scalar.add(out=den, in_=ps, add=1.0)
                    # rec = 1/(1+inh) on DVE (only engine with tensor
                    # reciprocal; approx_fast HW rel err matches exact);
                    # final multiply alternates Pool/DVE so neither engine
                    # serializes the 8-oc tail. rec lives in SBUF so the
                    # Pool multiply never touches PSUM.
                    rec = div_pool.tile([128, 512], F32, name="rec",
                                        tag="rec")
                    nc.vector.reciprocal_approx_fast(out=rec, in_=den)
                    nc.gpsimd.tensor_tensor(out=rv[:, oc, sl],
                                            in0=rv[:, oc, sl],
                                            in1=rec, op=ALU.mult)
                # one store per slice (1 MB), behind lcw on SP; the last
                # slice is split so the post-divide tail store is short
                if ns < FR // 512 - 1:
                    nc.sync.dma_start(out=odv[:, :, sl], in_=rv[:, :, sl])
                else:
                    nc.sync.dma_start(out=odv[:, :4, sl],
                                      in_=rv[:, :4, sl])
                    nc.sync.dma_start(out=odv[:, 4:, sl],
                                      in_=rv[:, 4:, sl])
            # conv1 row 3 goes here: x row 3 has landed by now, and LC
            # chunk 4 (first row-1 locations) needs out1 row 3
            if ck == 1 and "conv1" in stages:
                conv1_rows([3])


def _pow2_scale(maxabs, target=120.0):
    return 2.0 ** np.floor(np.log2(target / np.maximum(maxabs, 1e-30)))


def _prep_inputs(x, w1, g1, b1, m1, v1, lc_w, g2, b2, m2, v2,
                 w3, g3, b3, m3, v3, sigmas):
    """Host-side shard + layout prep. Returns (variant, per-core maps)."""
    f4 = np.float32
    x = np.asarray(x, f4)
    inv1 = (g1 / np.sqrt(v1 + EPS)).astype(f4)
    beta1 = (b1 - m1 * inv1).astype(f4)
    inv2 = (g2 / np.sqrt(v2 + EPS)).astype(f4)
    beta2 = (b2 - m2 * inv2).astype(f4)
    inv3 = (g3 / np.sqrt(v3 + EPS)).astype(f4)
    beta3 = (b3 - m3 * inv3).astype(f4)

    w1t = (np.asarray(w1, f4) * inv1[:, None]).T.reshape(CC1, 128, WID)
    w1t = np.ascontiguousarray(w1t).astype(NPBF16)
    w3t = (np.asarray(w3, f4) * inv3[:, None]).T.reshape(CCW, 128, COUT)
    w3t = np.ascontiguousarray(w3t).astype(NPBF16)

    # lc_w: (1,O,C,H,W,9) -> fp8 [h, w, p, (dk, ch, o)] with c = ch*128+p,
    # scaled per out-channel to a power of 2 (undone by BN2's act scale).
    lcw = np.asarray(lc_w[0], f4) * inv2[:, None, None, None, None]
    s2m = _pow2_scale(np.abs(lcw).max(axis=(1, 2, 3, 4)))   # (O,)
    lcw *= s2m[:, None, None, None, None]
    lcw = lcw.transpose(2, 3, 1, 4, 0)             # (H, W, C, 9, O)
    lcw = lcw.reshape(H, W, CCW, 128, 9, WID)      # (h, w, ch, p, dk, o)
    lcw = lcw.transpose(0, 1, 3, 4, 2, 5)          # (h, w, p, dk, ch, o)
    lcw = np.clip(lcw, -240.0, 240.0)
    lcw = np.ascontiguousarray(
        lcw.reshape(H, W, 128, 9 * 2 * WID)).astype(NPFP8)
    s2inv = (1.0 / s2m).astype(f4)

    # x bf16: (C, Hpad, W, N), rows zero-padded at both ends
    xt = np.zeros((CIN, H + 2, W, N), f4)
    xt[:, 1:H + 1] = x.transpose(1, 2, 3, 0)
    xtb = xt.astype(NPBF16)

    # inhibition mixing matrix g on host (fp32), then SVD -> low rank
    idx = np.arange(COUT)
    ci = np.abs(idx + 1.0 - (COUT // 2 + 1.0))
    dist = ci[(idx[None, :] - idx[:, None]) % COUT]          # (O, C)
    sig = np.maximum(np.asarray(sigmas, np.float64), 0.5)
    g = np.exp(-dist.astype(np.float64) ** 2 / (2.0 * sig ** 2)) / sig
    g = g / g.sum(axis=0)                                     # (O, C)
    U, S, Vt = np.linalg.svd(g)
    tail = float(S[RANK] / S[0]) if S.shape[0] > RANK else 0.0
    variant = "lr" if tail < 1e-3 else "dense"
    if variant == "lr":
        A = (U[:, :RANK] * S[:RANK]).astype(f4)               # (O, r)
        B = Vt[:RANK].astype(f4)                              # (r, C)
        ga = np.concatenate([A.T, np.ones((1, COUT), f4)])    # (r+1, O)
        gbt = B.T.reshape(CC3, 128, RANK).astype(f4)          # (cc,p,r)
    else:
        gbt = np.zeros((CC3, 128, RANK), f4)

    # packed bf16 consts: [w3t | gbt | I128 | I64] along the free dim
    eye64 = np.zeros((128, 64), f4)
    eye64[:64, :64] = np.eye(64, dtype=f4)
    cbf = np.concatenate(
        [w3t.transpose(1, 0, 2).reshape(128, CCW * COUT).astype(f4),
         gbt.transpose(1, 0, 2).reshape(128, CC3 * RANK),
         np.eye(128, dtype=f4),
         eye64],
        axis=1).astype(NPBF16)
    w1c = np.ascontiguousarray(
        w1t.transpose(1, 0, 2).reshape(128, CC1 * WID).astype(f4)
    ).astype(NPBF16)
    # packed f32 consts: [b1(2) b2(2) s2(2) b3(8)] as columns
    cf4 = np.concatenate(
        [beta1.reshape(CCW, 128).T, beta2.reshape(CCW, 128).T,
         s2inv.reshape(CCW, 128).T, beta3.reshape(CC3, 128).T],
        axis=1).astype(f4)
    com = {
        "w1c": w1c,
        "cbf": np.ascontiguousarray(cbf),
        "cf4": np.ascontiguousarray(cf4),
    }
    if variant == "lr":
        com["ga"] = np.ascontiguousarray(ga).astype(NPBF16)
    else:
        # device layout [c, o]: gd[cc][p, o] = g[o, cc*128+p]
        com["gd"] = np.ascontiguousarray(
            g.T.astype(f4).reshape(CC3, 128, COUT)).astype(NPBF16)

    in_maps = []
    for r in range(NCORES):
        r0 = r * RPC
        # x: (C, HLO, W, N) -> [128, (cc, h, w, n)]
        xbc = np.ascontiguousarray(
            xtb[:, r0:r0 + HLO].reshape(CC1, 128, XBF).transpose(1, 0, 2)
        ).reshape(128, CC1 * XBF)
        lw = np.ascontiguousarray(lcw[r0:r0 + RPC]).reshape(
            NLOC, 128, 9 * 2 * WID)
        if r == 0 or r == NCORES - 1:
            lw = lw.copy()
            if r == 0:           # row 0 locations: di=0 taps read row -1
                lw[0:W, :, 0:3 * 2 * WID] = 0
            if r == NCORES - 1:  # row 15 locations: di=2 taps read row 16
                lw[W:2 * W, :, 6 * 2 * WID:] = 0
        # group 4 locations per DMA chunk: [8, 128, 4*4608]
        lw = np.ascontiguousarray(
            lw.reshape(NCHUNK, LPC, 128, 9 * 2 * WID).transpose(0, 2, 1, 3)
        ).reshape(NCHUNK, 128, LPC * 9 * 2 * WID)
        in_maps.append(dict(com, xb=xbc, lcw=lw))
    return variant, in_maps


def _assemble(results):
    """results: per-core dicts with 'out' [128, CC3*FR] bf16 -> (N,C,H,W)"""
    full = np.empty((N, COUT, H, W), np.float32)
    for r, res in enumerate(results):
        o = np.asarray(res["out"]).astype(np.float32)
        o = o.reshape(128, CC3, RPC, W, N)
        # (p, cc, hl, j, n) -> (n, c=cc*128+p, h, w)
        o = o.transpose(4, 1, 0, 2, 3).reshape(N, COUT, RPC, W)
        full[:, :, r * RPC:(r + 1) * RPC, :] = o
    return full


_NC_CACHE = {}


def get_nc(ktimes: int = 1, variant: str = "lr", stages=ALL_STAGES):
    key = (ktimes, variant, tuple(stages))
    if key not in _NC_CACHE:
        _NC_CACHE[key] = _build_nc(ktimes, variant, stages)
    return _NC_CACHE[key]


def kernel(**inputs):
    variant, in_maps = _prep_inputs(**inputs)
    nc = get_nc(1, variant)
    res = run_bass_kernel_spmd(nc, in_maps, core_ids=list(range(NCORES)))
    return _assemble(res.results)


if __name__ == "__main__":
    rng = np.random.default_rng(0)
    ins = {
        "x": rng.standard_normal((N, CIN, H, W)).astype(np.float32),
        "w1": (rng.standard_normal((WID, CIN)).astype(np.float32) * 0.05),
        "g1": rng.random(WID).astype(np.float32),
        "b1": rng.standard_normal(WID).astype(np.float32) * 0.05,
        "m1": np.zeros(WID, np.float32),
        "v1": np.ones(WID, np.float32),
        "lc_w": rng.standard_normal((1, WID, WID, H, W, 9)).astype(
            np.float32) * 0.05,
        "g2": rng.random(WID).astype(np.float32),
        "b2": rng.standard_normal(WID).astype(np.float32) * 0.05,
        "m2": np.zeros(WID, np.float32),
        "v2": np.ones(WID, np.float32),
        "w3": rng.standard_normal((COUT, WID)).astype(np.float32) * 0.05,
        "g3": rng.random(COUT).astype(np.float32),
        "b3": rng.standard_normal(COUT).astype(np.float32) * 0.05,
        "m3": np.zeros(COUT, np.float32),
        "v3": np.ones(COUT, np.float32),
        "sigmas": rng.random(COUT).astype(np.float32) + COUT / 8.0,
    }
    out = kernel(**ins)
    print("out", out.shape, out.dtype, float(np.abs(out).max()))


# revision 23
# speedup vs baseline: 1.7921x; 1.7921x over previous
"""Trainium2 Bass kernel for nn_BrainBottleneckLocal (dense_cnn).

Sharding: spatial rows. H=16 rows are split 2-per-core across 8 NeuronCores;
every layer is then core-local (the LC weight is per-location, so the 604 MB
lc_w tensor splits 8x by row — the dominant DMA stream).

Single-shot latency is DMA-bound: the per-core DMA budget is ~33 MB
(lcw fp8 18.9 + x bf16 8.4 + consts 1.2 + out bf16 4.2) against one
~360 GB/s DMA-engine pool => ~91 us floor. Per-queue DMA issue latency is
~2.7 us, so the stream is packed into FEW HUGE DMAs and spread over the
three issue queues:
  - SP: w1 weights, then the lcw stream as 4 chunks (4.7 MB each), then the
    4 output slice-stores (traced later => behind lcw in the FIFO, so they
    drain during the compute tail without delaying the weight stream).
  - Act: x in 2 DMAs (rows 0-2: 6.3 MB, row 3: 2.1 MB).
  - Pool/SWDGE: small consts (cbf/cf4/ga), done in the first ~10 us.
The trace interleaves each LC chunk with its dependent conv3/inhibition
512-slice so the PE FIFO pipelines the tail (chunk k's slice runs while
chunk k+1 streams in).

Per-core pipeline (free-dim layout is (h, w, n) everywhere):
  1. conv1x1 #1 + BN1 + ReLU on the core's 2 rows plus a 1-row halo each side
     (4 rows, boundary rows zero-padded by the host). Output is written
     straight to fp8-e4m3 (the LC input quantization).
  2. locally-connected 3x3 + BN2 + ReLU: fp8 weights (per-out-channel pow2
     scale folded in, undone by BN2's per-partition activation scale) and fp8
     patches via the tensor engine's DoubleRow perf mode (2 fp8 MACs/cell).
  3. conv1x1 #2 + BN3, residual add (PSUM-accumulated identity matmul vs the
     bf16 conv1 input tile), ReLU -> resb bf16.
  4. opponent inhibition through a low-rank factorization of the mixing
     matrix g (host-side SVD; sigma ~ C/8 makes g numerically rank <~16):
     inh = A @ (B @ resb), then out = resb / (1 + inh), stored bf16.
Matmuls accumulate in fp32 PSUM. BN scales are folded into weights on the
host; BN biases apply via per-partition activation bias. All cores run an
identical program; only per-core data differs (boundary handling = zeroed
LC taps).
"""

from contextlib import ExitStack

import numpy as np

import concourse.bacc as bacc
import concourse.mybir as mybir
import concourse.tile as tile
from concourse.bass_utils import run_bass_kernel_spmd

F32 = mybir.dt.float32
BF16 = mybir.dt.bfloat16
FP8 = mybir.dt.float8e4
NPBF16 = mybir.dt.np(BF16)
NPFP8 = mybir.dt.np(FP8)

EPS = 1e-5
N, CIN, H, W = 64, 1024, 16, 16
WID, COUT = 256, 1024
NCORES = 8
RPC = H // NCORES          # rows per core = 2
HLO = RPC + 2              # rows incl halo = 4
WP = W + 2                 # padded width = 18
NLOC = RPC * W             # LC locations per core = 32
CC1 = CIN // 128           # 8
CCW = WID // 128           # 2
CC3 = COUT // 128          # 8
FR = RPC * W * N           # free size of per-core row block = 2048
RANK = 16                  # low-rank size for the inhibition mixing matrix
NCHUNK = 8                 # lcw stream chunks (4 locations each)
LPC = NLOC // NCHUNK       # locations per chunk = 8
XBF = HLO * W * N          # per-conv1-chunk free size = 4096
# packed bf16 consts width: w3t | gbt | 128x128 identity | 64x64 identity
CBF_X = CCW * COUT + CC3 * RANK + 128 + 64
AF = mybir.ActivationFunctionType
ALU = mybir.AluOpType
DR = mybir.MatmulPerfMode.DoubleRow


def _declare_drams(nc, variant):
    ap = {}
    # x, (p, (cc, h, w, n)) so conv1 chunks are column ranges of one tile
    ap["xb"] = nc.dram_tensor("xb", [128, CC1 * XBF], BF16,
                              kind="ExternalInput").ap()
    # lcw stream: 8 chunks x 4 locations, 18.4KB DMA lines
    ap["lcw"] = nc.dram_tensor("lcw", [NCHUNK, 128, LPC * 9 * 2 * WID], FP8,
                               kind="ExternalInput").ap()
    # conv1 weights alone (first thing conv1 needs)
    ap["w1c"] = nc.dram_tensor("w1c", [128, CC1 * WID], BF16,
                               kind="ExternalInput").ap()
    # packed constants:
    #   cbf: [w3t (2*1024) | gbt (8*16) | I128 | I64] bf16, 128-part
    #   cf4: [b1 (2) | b2 (2) | s2 (2) | b3 (8)] f32 columns, 128-part
    ap["cbf"] = nc.dram_tensor("cbf", [128, CBF_X], BF16,
                               kind="ExternalInput").ap()
    ap["cf4"] = nc.dram_tensor("cf4", [128, 14], F32,
                               kind="ExternalInput").ap()
    if variant == "lr":
        # row RANK of ga is all-ones: stage-2 matmul then yields 1 + inh
        ap["ga"] = nc.dram_tensor("ga", [RANK + 1, COUT], BF16,
                                  kind="ExternalInput").ap()
    else:
        ap["gd"] = nc.dram_tensor("gd", [CC3, 128, COUT], BF16,
                                  kind="ExternalInput").ap()
    # out, (p, (oc, hl, j, n)) so a 512-slice store spans all 8 oc chunks
    ap["out"] = nc.dram_tensor("out", [128, CC3 * FR], BF16,
                               kind="ExternalOutput").ap()
    return ap


ALL_STAGES = ("conv1", "lcdma", "lcmm", "conv3", "inhib")


def _build_nc(ktimes: int = 1, variant: str = "lr", stages=ALL_STAGES):
    nc = bacc.Bacc("TRN2", target_bir_lowering=False, debug=False,
                   num_devices=NCORES)
    ap = _declare_drams(nc, variant)
    with tile.TileContext(nc) as tc:
        if ktimes == 1:
            _trace_kernel(tc, nc, ap, variant, stages)
        else:
            with tc.For_i(0, ktimes, 1):
                _trace_kernel(tc, nc, ap, variant, stages)
    nc.compile()
    return nc


def _trace_kernel(tc, nc, ap, variant="lr", stages=ALL_STAGES):
    with ExitStack() as ctx:
        persist = ctx.enter_context(tc.tile_pool(name="persist", bufs=1))
        psum = ctx.enter_context(
            tc.tile_pool(name="psum", bufs=3, space="PSUM"))

        # ---- SP queue: x rows 0-1, even lcw chunks, stores ---------------
        # (w1c goes via SWDGE so x leads the HWDGE queues)
        w1c_t = persist.tile([128, CC1 * WID], BF16, name="w1c", tag="w1c")
        nc.gpsimd.dma_start(out=w1c_t, in_=ap["w1c"])
        xball = persist.tile([128, CC1 * XBF], BF16, name="xball", tag="xb")
        xv = xball.rearrange("p (c f) -> p c f", c=CC1)
        xdv = ap["xb"].rearrange("p (c f) -> p c f", c=CC1)
        nc.sync.dma_start(out=xv[:, :, :W * N], in_=xdv[:, :, :W * N])
        nc.sync.dma_start(out=xv[:, :, W * N:2 * W * N],
                          in_=xdv[:, :, W * N:2 * W * N])
        # ---- Act queue: x row 2 first, then odd lcw chunks, then row 3 ----
        nc.scalar.dma_start(out=xv[:, :, 2 * W * N:3 * W * N],
                            in_=xdv[:, :, 2 * W * N:3 * W * N])
        lcw_pool = ctx.enter_context(tc.tile_pool(name="lcwp", bufs=4))
        lw_tiles = []
        if "lcdma" in stages:
            for ck in range(NCHUNK):
                lw = lcw_pool.tile([128, LPC * 9 * 2 * WID], FP8,
                                   name="lcw_t", tag="lcw")
                eng = (nc.sync, nc.scalar)[ck % 2]
                eng.dma_start(out=lw, in_=ap["lcw"][ck])
                lw_tiles.append(lw)
        # x row 3 behind the odd chunks: conv1 h=3 is traced after slice 1,
        # by which time this has landed
        nc.scalar.dma_start(out=xv[:, :, 3 * W * N:],
                            in_=xdv[:, :, 3 * W * N:])

        # ---- Pool/SWDGE queue: small consts -------------------------------
        cbf_t = persist.tile([128, CBF_X], BF16, name="cbf", tag="cbf")
        nc.gpsimd.dma_start(out=cbf_t, in_=ap["cbf"])
        cf4_t = persist.tile([128, 14], F32, name="cf4", tag="cf4")
        nc.gpsimd.dma_start(out=cf4_t, in_=ap["cf4"])

        w1t_t = [w1c_t[:, cc * WID:(cc + 1) * WID] for cc in range(CC1)]
        off = 0
        w3t_t = [cbf_t[:, off + oc * COUT:off + (oc + 1) * COUT]
                 for oc in range(CCW)]
        off += CCW * COUT
        gbt_t = [cbf_t[:, off + cc * RANK:off + (cc + 1) * RANK]
                 for cc in range(CC3)]
        off += CC3 * RANK
        i128_t = cbf_t[:, off:off + 128]
        ident_t = cbf_t[0:64, off + 128:off + 128 + 64]
        b1_t = [cf4_t[:, c:c + 1] for c in range(CCW)]
        b2_t = [cf4_t[:, 2 + c:3 + c] for c in range(CCW)]
        s2_t = [cf4_t[:, 4 + c:5 + c] for c in range(CCW)]
        b3_t = [cf4_t[:, 6 + c:7 + c] for c in range(CC3)]
        if variant == "lr":
            ga_t = persist.tile([RANK + 1, COUT], BF16, name="ga", tag="ga")
            nc.gpsimd.dma_start(out=ga_t, in_=ap["ga"])
        else:
            gd_t = []
            for cc in range(CC3):
                t = persist.tile([128, COUT], BF16, name=f"gd_{cc}",
                                 tag=f"gd{cc}")
                nc.gpsimd.dma_start(out=t, in_=ap["gd"][cc])
                gd_t.append(t)

        out2_t = [persist.tile([128, FR], BF16, name=f"out2_{oc}",
                               tag=f"out2{oc}") for oc in range(CCW)]
        # single resb tile so one DMA stores a 512-slice across all 8 oc
        resb = persist.tile([128, CC3 * FR], BF16, name="resb", tag="resb")
        rv = resb.rearrange("p (c f) -> p c f", c=CC3)
        odv = ap["out"].rearrange("p (c f) -> p c f", c=CC3)

        # out1 fp8, padded width: [128, (c2, h4, w18, n64)], pad cols zeroed
        out1q = persist.tile([128, CCW * HLO * WP * N], FP8, name="out1q",
                             tag="out1q")
        o1v = out1q.rearrange("p (c h w n) -> p c h w n",
                              c=CCW, h=HLO, w=WP)
        nc.gpsimd.memset(o1v[:, :, :, 0, :], 0.0)
        nc.gpsimd.memset(o1v[:, :, :, W + 1, :], 0.0)

        lct_pool = ctx.enter_context(tc.tile_pool(name="lctp", bufs=2))
        div_pool = ctx.enter_context(tc.tile_pool(name="divp", bufs=4))
        yb_t = None
        if variant == "lr":
            # moving operand of inhibition stage 2; row RANK stays 1.0.
            yb_t = [persist.tile([RANK + 1, 512], BF16, name=f"yb{i}",
                                 tag=f"yb{i}") for i in range(2)]
            for t in yb_t:
                nc.gpsimd.memset(t, 1.0)

        def conv1_rows(rows):
            # conv1x1 #1 + BN1 + ReLU -> padded fp8 out1 (skip W-pad cols)
            for h in rows:
                for oc in range(CCW):
                    for q in range(2):
                        ps = psum.tile([128, 512], F32, name="ps1", tag="a",
                                       bufs=2)
                        base = h * (W * N) + q * 512
                        for cc in range(CC1):
                            nc.tensor.matmul(
                                ps,
                                w1t_t[cc][:, oc * 128:(oc + 1) * 128],
                                xball[:,
                                      cc * XBF + base:cc * XBF + base + 512],
                                start=(cc == 0), stop=(cc == CC1 - 1))
                        nc.scalar.activation(
                            out=o1v[:, oc, h, 1 + 8 * q:9 + 8 * q, :],
                            in_=ps.rearrange("p (w n) -> p w n", n=N),
                            func=AF.Relu, bias=b1_t[oc], scale=1.0)

        if "conv1" in stages:
            conv1_rows(range(3))   # rows 0-2; row 3 traced after slice 1

        if "lcmm" not in stages:
            for oc in range(CCW):
                nc.gpsimd.memset(out2_t[oc], 0.01)
        lw_shared = None
        if "lcdma" not in stages and "lcmm" in stages:
            lw_shared = persist.tile([128, LPC * 9 * 2 * WID], FP8,
                                     name="lw_shared", tag="lws")
            nc.gpsimd.memset(lw_shared, 0.01)
        if "conv3" not in stages:
            nc.gpsimd.memset(resb, 0.01)

        for ck in range(NCHUNK):
            # -- LC chunk: 4 locations -> out2 cols [ck*256, ck*256+256) --
            if "lcmm" in stages:
                lw = lw_tiles[ck] if "lcdma" in stages else lw_shared
                lwv = lw.rearrange("p (l dk c o) -> p l dk c o",
                                   l=LPC, dk=9, c=2)
                pst_all = psum.tile([128, CCW * LPC * N], BF16, name="pst",
                                    tag="tp", bufs=1)
                pst = [pst_all[:, oc * LPC * N:(oc + 1) * LPC * N]
                       for oc in range(CCW)]
                for lp in range(LPC // 2):
                    # two locations accumulate into one PSUM bank so the
                    # psum->sbuf copy is one big transfer, alternating
                    # between DVE and Act so neither paces the LC loop
                    ps2 = psum.tile([64, 2 * WID], F32, name="ps2",
                                    tag="lc", bufs=2)
                    for half in range(2):
                        li = lp * 2 + half
                        loc = ck * LPC + li
                        hl, j = divmod(loc, W)
                        po = half * WID
                        for dk in range(9):
                            di, dj = divmod(dk, 3)
                            nc.tensor.matmul(
                                ps2[:, po:po + WID],
                                o1v[:, :, hl + di, j + dj, :],
                                lwv[:, li, dk],
                                start=(dk == 0), stop=(dk == 8),
                                perf_mode=DR)
                    tmpb = lct_pool.tile([64, 2 * WID], BF16, name="tmpb",
                                         tag="tmpb")
                    if lp % 2 == 0:
                        nc.vector.tensor_copy(out=tmpb, in_=ps2)
                    else:
                        nc.scalar.activation(out=tmpb, in_=ps2,
                                             func=AF.Copy, scale=1.0)
                    for half in range(2):
                        li = lp * 2 + half
                        for oc in range(CCW):
                            hb = half * WID + oc * 128
                            nc.tensor.transpose(
                                pst[oc][:, li * N:(li + 1) * N],
                                tmpb[:, hb:hb + 128], ident_t)
                for oc in range(CCW):
                    nc.scalar.activation(
                        out=out2_t[oc][:, ck * LPC * N:(ck + 1) * LPC * N],
                        in_=pst[oc], func=AF.Relu, bias=b2_t[oc],
                        scale=s2_t[oc])

            if ck % 2 == 0:
                continue
            # -- slice ns = ck//2: conv3 + BN3 + residual + ReLU, inhibition,
            # divide, store --
            ns = ck // 2
            sl = slice(ns * 512, ns * 512 + 512)
            for oc3 in range(CC3 if "conv3" in stages else 0):
                ps = psum.tile([128, 512], F32, name="ps3", tag="a", bufs=2)
                for oc in range(CCW):
                    nc.tensor.matmul(
                        ps, w3t_t[oc][:, oc3 * 128:(oc3 + 1) * 128],
                        out2_t[oc][:, sl],
                        start=(oc == 0), stop=False)
                # residual add rides the PSUM accumulator: ps += I @ x
                rb = oc3 * XBF + W * N + ns * 512
                nc.tensor.matmul(ps, i128_t, xball[:, rb:rb + 512],
                                 start=False, stop=True)
                # resb = relu(ps + beta3); alternate Act/DVE so the 8
                # BN3 ops do not serialize on one engine ahead of yps
                if oc3 % 2 == 0:
                    nc.scalar.activation(out=rv[:, oc3, sl], in_=ps,
                                         func=AF.Relu, bias=b3_t[oc3],
                                         scale=1.0)
                else:
                    nc.vector.tensor_scalar(
                        out=rv[:, oc3, sl], in0=ps, scalar1=b3_t[oc3],
                        scalar2=0.0, op0=ALU.add, op1=ALU.max)
            if "inhib" in stages:
                if variant == "lr":
                    yps = psum.tile([RANK, 512], F32, name="yps", tag="lc",
                                    bufs=2)
                    for cc in range(CC3):
                        nc.tensor.matmul(yps, gbt_t[cc], rv[:, cc, sl],
                                         start=(cc == 0),
                                         stop=(cc == CC3 - 1))
                    yb = yb_t[ns % 2]
                    nc.vector.tensor_copy(out=yb[:RANK], in_=yps)
                for oc in range(CC3):
                    ps = psum.tile([128, 512], F32, name="ps4", tag="s4",
                                   bufs=3)
                    if variant == "lr":
                        # lhsT row RANK is ones, yb row RANK is ones:
                        # psum = inh + 1 directly
                        nc.tensor.matmul(
                            ps, ga_t[:, oc * 128:(oc + 1) * 128],
                            yb, start=True, stop=True)
                        den = ps
                    else:
                        for cc in range(CC3):
                            nc.tensor.matmul(
                                ps, gd_t[cc][:, oc * 128:(oc + 1) * 128],
                                rv[:, cc, sl],
                                start=(cc == 0), stop=(cc == CC3 - 1))
                        den = div_pool.tile([128, 512], F32, name="den",
                                            tag="den")
                        nc.scalar.add(out=den, in_=ps, add=1.0)
                    # rec = 1/(1+inh) on DVE (only engine with tensor
                    # reciprocal; approx_fast HW rel err matches exact);
                    # final multiply alternates Pool/DVE so neither engine
                    # serializes the 8-oc tail. rec lives in SBUF so the
                    # Pool multiply never touches PSUM.
                    rec = div_pool.tile([128, 512], F32, name="rec",
                                        tag="rec")
                    nc.vector.reciprocal_approx_fast(out=rec, in_=den)
                    if ns < FR // 512 - 1:
                        feng = nc.gpsimd
                    else:
                        feng = (nc.gpsimd, nc.vector)[oc % 2]
                    feng.tensor_tensor(out=rv[:, oc, sl],
                                       in0=rv[:, oc, sl],
                                       in1=rec, op=ALU.mult)
                # one store per slice (1 MB), behind lcw on SP; the last
                # slice is split so the post-divide tail store is short
                if ns < FR // 512 - 1:
                    nc.sync.dma_start(out=odv[:, :, sl], in_=rv[:, :, sl])
                else:
                    nc.sync.dma_start(out=odv[:, :4, sl],
                                      in_=rv[:, :4, sl])
                    nc.sync.dma_start(out=odv[:, 4:, sl],
                                      in_=rv[:, 4:, sl])
            # conv1 row 3 goes here: x row 3 has landed by now, and LC
            # chunk 4 (first row-1 locations) needs out1 row 3
            if ck == 1 and "conv1" in stages:
                conv1_rows([3])


def _pow2_scale(maxabs, target=120.0):
    return 2.0 ** np.floor(np.log2(target / np.maximum(maxabs, 1e-30)))


def _prep_inputs(x, w1, g1, b1, m1, v1, lc_w, g2, b2, m2, v2,
                 w3, g3, b3, m3, v3, sigmas):
    """Host-side shard + layout prep. Returns (variant, per-core maps)."""
    f4 = np.float32
    x = np.asarray(x, f4)
    inv1 = (g1 / np.sqrt(v1 + EPS)).astype(f4)
    beta1 = (b1 - m1 * inv1).astype(f4)
    inv2 = (g2 / np.sqrt(v2 + EPS)).astype(f4)
    beta2 = (b2 - m2 * inv2).astype(f4)
    inv3 = (g3 / np.sqrt(v3 + EPS)).astype(f4)
    beta3 = (b3 - m3 * inv3).astype(f4)

    w1t = (np.asarray(w1, f4) * inv1[:, None]).T.reshape(CC1, 128, WID)
    w1t = np.ascontiguousarray(w1t).astype(NPBF16)
    w3t = (np.asarray(w3, f4) * inv3[:, None]).T.reshape(CCW, 128, COUT)
    w3t = np.ascontiguousarray(w3t).astype(NPBF16)

    # lc_w: (1,O,C,H,W,9) -> fp8 [h, w, p, (dk, ch, o)] with c = ch*128+p,
    # scaled per out-channel to a power of 2 (undone by BN2's act scale).
    lcw = np.asarray(lc_w[0], f4) * inv2[:, None, None, None, None]
    s2m = _pow2_scale(np.abs(lcw).max(axis=(1, 2, 3, 4)))   # (O,)
    lcw *= s2m[:, None, None, None, None]
    lcw = lcw.transpose(2, 3, 1, 4, 0)             # (H, W, C, 9, O)
    lcw = lcw.reshape(H, W, CCW, 128, 9, WID)      # (h, w, ch, p, dk, o)
    lcw = lcw.transpose(0, 1, 3, 4, 2, 5)          # (h, w, p, dk, ch, o)
    lcw = np.clip(lcw, -240.0, 240.0)
    lcw = np.ascontiguousarray(
        lcw.reshape(H, W, 128, 9 * 2 * WID)).astype(NPFP8)
    s2inv = (1.0 / s2m).astype(f4)

    # x bf16: (C, Hpad, W, N), rows zero-padded at both ends
    xt = np.zeros((CIN, H + 2, W, N), f4)
    xt[:, 1:H + 1] = x.transpose(1, 2, 3, 0)
    xtb = xt.astype(NPBF16)

    # inhibition mixing matrix g on host (fp32), then SVD -> low rank
    idx = np.arange(COUT)
    ci = np.abs(idx + 1.0 - (COUT // 2 + 1.0))
    dist = ci[(idx[None, :] - idx[:, None]) % COUT]          # (O, C)
    sig = np.maximum(np.asarray(sigmas, np.float64), 0.5)
    g = np.exp(-dist.astype(np.float64) ** 2 / (2.0 * sig ** 2)) / sig
    g = g / g.sum(axis=0)                                     # (O, C)
    U, S, Vt = np.linalg.svd(g)
    tail = float(S[RANK] / S[0]) if S.shape[0] > RANK else 0.0
    variant = "lr" if tail < 1e-3 else "dense"
    if variant == "lr":
        A = (U[:, :RANK] * S[:RANK]).astype(f4)               # (O, r)
        B = Vt[:RANK].astype(f4)                              # (r, C)
        ga = np.concatenate([A.T, np.ones((1, COUT), f4)])    # (r+1, O)
        gbt = B.T.reshape(CC3, 128, RANK).astype(f4)          # (cc,p,r)
    else:
        gbt = np.zeros((CC3, 128, RANK), f4)

    # packed bf16 consts: [w3t | gbt | I128 | I64] along the free dim
    eye64 = np.zeros((128, 64), f4)
    eye64[:64, :64] = np.eye(64, dtype=f4)
    cbf = np.concatenate(
        [w3t.transpose(1, 0, 2).reshape(128, CCW * COUT).astype(f4),
         gbt.transpose(1, 0, 2).reshape(128, CC3 * RANK),
         np.eye(128, dtype=f4),
         eye64],
        axis=1).astype(NPBF16)
    w1c = np.ascontiguousarray(
        w1t.transpose(1, 0, 2).reshape(128, CC1 * WID).astype(f4)
    ).astype(NPBF16)
    # packed f32 consts: [b1(2) b2(2) s2(2) b3(8)] as columns
    cf4 = np.concatenate(
        [beta1.reshape(CCW, 128).T, beta2.reshape(CCW, 128).T,
         s2inv.reshape(CCW, 128).T, beta3.reshape(CC3, 128).T],
        axis=1).astype(f4)
    com = {
        "w1c": w1c,
        "cbf": np.ascontiguousarray(cbf),
        "cf4": np.ascontiguousarray(cf4),
    }
    if variant == "lr":
        com["ga"] = np.ascontiguousarray(ga).astype(NPBF16)
    else:
        # device layout [c, o]: gd[cc][p, o] = g[o, cc*128+p]
        com["gd"] = np.ascontiguousarray(
            g.T.astype(f4).reshape(CC3, 128, COUT)).astype(NPBF16)

    in_maps = []
    for r in range(NCORES):
        r0 = r * RPC
        # x: (C, HLO, W, N) -> [128, (cc, h, w, n)]
        xbc = np.ascontiguousarray(
            xtb[:, r0:r0 + HLO].reshape(CC1, 128, XBF).transpose(1, 0, 2)
        ).reshape(128, CC1 * XBF)
        lw = np.ascontiguousarray(lcw[r0:r0 + RPC]).reshape(
            NLOC, 128, 9 * 2 * WID)
        if r == 0 or r == NCORES - 1:
            lw = lw.copy()
            if r == 0:           # row 0 locations: di=0 taps read row -1
                lw[0:W, :, 0:3 * 2 * WID] = 0
            if r == NCORES - 1:  # row 15 locations: di=2 taps read row 16
                lw[W:2 * W, :, 6 * 2 * WID:] = 0
        # group 4 locations per DMA chunk: [8, 128, 4*4608]
        lw = np.ascontiguousarray(
            lw.reshape(NCHUNK, LPC, 128, 9 * 2 * WID).transpose(0, 2, 1, 3)
        ).reshape(NCHUNK, 128, LPC * 9 * 2 * WID)
        in_maps.append(dict(com, xb=xbc, lcw=lw))
    return variant, in_maps


def _assemble(results):
    """results: per-core dicts with 'out' [128, CC3*FR] bf16 -> (N,C,H,W)"""
    full = np.empty((N, COUT, H, W), np.float32)
    for r, res in enumerate(results):
        o = np.asarray(res["out"]).astype(np.float32)
        o = o.reshape(128, CC3, RPC, W, N)
        # (p, cc, hl, j, n) -> (n, c=cc*128+p, h, w)
        o = o.transpose(4, 1, 0, 2, 3).reshape(N, COUT, RPC, W)
        full[:, :, r * RPC:(r + 1) * RPC, :] = o
    return full


_NC_CACHE = {}


def get_nc(ktimes: int = 1, variant: str = "lr", stages=ALL_STAGES):
    key = (ktimes, variant, tuple(stages))
    if key not in _NC_CACHE:
        _NC_CACHE[key] = _build_nc(ktimes, variant, stages)
    return _NC_CACHE[key]


def kernel(**inputs):
    variant, in_maps = _prep_inputs(**inputs)
    nc = get_nc(1, variant)
    res = run_bass_kernel_spmd(nc, in_maps, core_ids=list(range(NCORES)))
    return _assemble(res.results)


if __name__ == "__main__":
    rng = np.random.default_rng(0)
    ins = {
        "x": rng.standard_normal((N, CIN, H, W)).astype(np.float32),
        "w1": (rng.standard_normal((WID, CIN)).astype(np.float32) * 0.05),
        "g1": rng.random(WID).astype(np.float32),
        "b1": rng.standard_normal(WID).astype(np.float32) * 0.05,
        "m1": np.zeros(WID, np.float32),
        "v1": np.ones(WID, np.float32),
        "lc_w": rng.standard_normal((1, WID, WID, H, W, 9)).astype(
            np.float32) * 0.05,
        "g2": rng.random(WID).astype(np.float32),
        "b2": rng.standard_normal(WID).astype(np.float32) * 0.05,
        "m2": np.zeros(WID, np.float32),
        "v2": np.ones(WID, np.float32),
        "w3": rng.standard_normal((COUT, WID)).astype(np.float32) * 0.05,
        "g3": rng.random(COUT).astype(np.float32),
        "b3": rng.standard_normal(COUT).astype(np.float32) * 0.05,
        "m3": np.zeros(COUT, np.float32),
        "v3": np.ones(COUT, np.float32),
        "sigmas": rng.random(COUT).astype(np.float32) + COUT / 8.0,
    }
    out = kernel(**ins)
    print("out", out.shape, out.dtype, float(np.abs(out).max()))


# revision 24
# speedup vs baseline: 2.4566x; 1.3708x over previous
"""Trainium2 Bass kernel for nn_BrainBottleneckLocal (dense_cnn).

Sharding: spatial rows. H=16 rows are split 2-per-core across 8 NeuronCores;
every layer is then core-local (the LC weight is per-location, so the 604 MB
lc_w tensor splits 8x by row — the dominant DMA stream).

Single-shot latency is DMA-bound: the per-core DMA budget is ~33 MB
(lcw fp8 18.9 + x bf16 8.4 + consts 1.2 + out bf16 4.2) against one
~360 GB/s DMA-engine pool => ~91 us floor. Per-queue DMA issue latency is
~2.7 us, so the stream is packed into FEW HUGE DMAs and spread over the
three issue queues:
  - SP: w1 weights, then the lcw stream as 4 chunks (4.7 MB each), then the
    4 output slice-stores (traced later => behind lcw in the FIFO, so they
    drain during the compute tail without delaying the weight stream).
  - Act: x in 2 DMAs (rows 0-2: 6.3 MB, row 3: 2.1 MB).
  - Pool/SWDGE: small consts (cbf/cf4/ga), done in the first ~10 us.
The trace interleaves each LC chunk with its dependent conv3/inhibition
512-slice so the PE FIFO pipelines the tail (chunk k's slice runs while
chunk k+1 streams in).

Per-core pipeline (free-dim layout is (h, w, n) everywhere):
  1. conv1x1 #1 + BN1 + ReLU on the core's 2 rows plus a 1-row halo each side
     (4 rows, boundary rows zero-padded by the host). Output is written
     straight to fp8-e4m3 (the LC input quantization).
  2. locally-connected 3x3 + BN2 + ReLU: fp8 weights (per-out-channel pow2
     scale folded in, undone by BN2's per-partition activation scale) and fp8
     patches via the tensor engine's DoubleRow perf mode (2 fp8 MACs/cell).
  3. conv1x1 #2 + BN3, residual add (PSUM-accumulated identity matmul vs the
     bf16 conv1 input tile), ReLU -> resb bf16.
  4. opponent inhibition through a low-rank factorization of the mixing
     matrix g (host-side SVD; sigma ~ C/8 makes g numerically rank <~16):
     inh = A @ (B @ resb), then out = resb / (1 + inh), stored bf16.
Matmuls accumulate in fp32 PSUM. BN scales are folded into weights on the
host; BN biases apply via per-partition activation bias. All cores run an
identical program; only per-core data differs (boundary handling = zeroed
LC taps).
"""

from contextlib import ExitStack

import numpy as np

import concourse.bacc as bacc
import concourse.mybir as mybir
import concourse.tile as tile
from concourse.bass_utils import run_bass_kernel_spmd

F32 = mybir.dt.float32
BF16 = mybir.dt.bfloat16
FP8 = mybir.dt.float8e4
NPBF16 = mybir.dt.np(BF16)
NPFP8 = mybir.dt.np(FP8)

EPS = 1e-5
N, CIN, H, W = 64, 1024, 16, 16
WID, COUT = 256, 1024
NCORES = 8
RPC = H // NCORES          # rows per core = 2
HLO = RPC + 2              # rows incl halo = 4
WP = W + 2                 # padded width = 18
NLOC = RPC * W             # LC locations per core = 32
CC1 = CIN // 128           # 8
CCW = WID // 128           # 2
CC3 = COUT // 128          # 8
FR = RPC * W * N           # free size of per-core row block = 2048
RANK = 16                  # low-rank size for the inhibition mixing matrix
NCHUNK = 8                 # lcw stream chunks (4 locations each)
LPC = NLOC // NCHUNK       # locations per chunk = 8
XBF = HLO * W * N          # per-conv1-chunk free size = 4096
# packed bf16 consts width: w3t | gbt | 128x128 identity | 64x64 identity
CBF_X = CCW * COUT + CC3 * RANK + 128 + 64
AF = mybir.ActivationFunctionType
ALU = mybir.AluOpType
DR = mybir.MatmulPerfMode.DoubleRow


def _declare_drams(nc, variant):
    ap = {}
    # x, (p, (cc, h, w, n)) so conv1 chunks are column ranges of one tile
    ap["xb"] = nc.dram_tensor("xb", [128, CC1 * XBF], BF16,
                              kind="ExternalInput").ap()
    # lcw stream: 8 chunks x 4 locations, 18.4KB DMA lines
    ap["lcw"] = nc.dram_tensor("lcw", [NCHUNK, 128, LPC * 9 * 2 * WID], FP8,
                               kind="ExternalInput").ap()
    # conv1 weights alone (first thing conv1 needs)
    ap["w1c"] = nc.dram_tensor("w1c", [128, CC1 * WID], BF16,
                               kind="ExternalInput").ap()
    # packed constants:
    #   cbf: [w3t (2*1024) | gbt (8*16) | I128 | I64] bf16, 128-part
    #   cf4: [b1 (2) | b2 (2) | s2 (2) | b3 (8)] f32 columns, 128-part
    ap["cbf"] = nc.dram_tensor("cbf", [128, CBF_X], BF16,
                               kind="ExternalInput").ap()
    ap["cf4"] = nc.dram_tensor("cf4", [128, 14], F32,
                               kind="ExternalInput").ap()
    if variant == "lr":
        # row RANK of ga is all-ones: stage-2 matmul then yields 1 + inh
        ap["ga"] = nc.dram_tensor("ga", [RANK + 1, COUT], BF16,
                                  kind="ExternalInput").ap()
    else:
        ap["gd"] = nc.dram_tensor("gd", [CC3, 128, COUT], BF16,
                                  kind="ExternalInput").ap()
    # out, (p, (oc, hl, j, n)) so a 512-slice store spans all 8 oc chunks
    ap["out"] = nc.dram_tensor("out", [128, CC3 * FR], BF16,
                               kind="ExternalOutput").ap()
    return ap


ALL_STAGES = ("conv1", "lcdma", "lcmm", "conv3", "inhib")


def _build_nc(ktimes: int = 1, variant: str = "lr", stages=ALL_STAGES):
    nc = bacc.Bacc("TRN2", target_bir_lowering=False, debug=False,
                   num_devices=NCORES)
    ap = _declare_drams(nc, variant)
    with tile.TileContext(nc) as tc:
        if ktimes == 1:
            _trace_kernel(tc, nc, ap, variant, stages)
        else:
            with tc.For_i(0, ktimes, 1):
                _trace_kernel(tc, nc, ap, variant, stages)
    nc.compile()
    return nc


def _trace_kernel(tc, nc, ap, variant="lr", stages=ALL_STAGES):
    with ExitStack() as ctx:
        persist = ctx.enter_context(tc.tile_pool(name="persist", bufs=1))
        psum = ctx.enter_context(
            tc.tile_pool(name="psum", bufs=3, space="PSUM"))

        # ---- SP queue: x rows 0-1, even lcw chunks, stores ---------------
        # (w1c goes via SWDGE so x leads the HWDGE queues)
        w1c_t = persist.tile([128, CC1 * WID], BF16, name="w1c", tag="w1c")
        nc.gpsimd.dma_start(out=w1c_t, in_=ap["w1c"])
        xball = persist.tile([128, CC1 * XBF], BF16, name="xball", tag="xb")
        xv = xball.rearrange("p (c f) -> p c f", c=CC1)
        xdv = ap["xb"].rearrange("p (c f) -> p c f", c=CC1)
        nc.sync.dma_start(out=xv[:, :, :W * N], in_=xdv[:, :, :W * N])
        nc.sync.dma_start(out=xv[:, :, W * N:2 * W * N],
                          in_=xdv[:, :, W * N:2 * W * N])
        # ---- Act queue: x row 2 first, then odd lcw chunks, then row 3 ----
        nc.scalar.dma_start(out=xv[:, :, 2 * W * N:3 * W * N],
                            in_=xdv[:, :, 2 * W * N:3 * W * N])
        lcw_pool = ctx.enter_context(tc.tile_pool(name="lcwp", bufs=4))
        lw_tiles = []
        if "lcdma" in stages:
            for ck in range(NCHUNK):
                lw = lcw_pool.tile([128, LPC * 9 * 2 * WID], FP8,
                                   name="lcw_t", tag="lcw")
                eng = (nc.sync, nc.scalar)[ck % 2]
                eng.dma_start(out=lw, in_=ap["lcw"][ck])
                lw_tiles.append(lw)
        # x row 3 behind the odd chunks: conv1 h=3 is traced after slice 1,
        # by which time this has landed
        nc.scalar.dma_start(out=xv[:, :, 3 * W * N:],
                            in_=xdv[:, :, 3 * W * N:])

        # ---- Pool/SWDGE queue: small consts -------------------------------
        cbf_t = persist.tile([128, CBF_X], BF16, name="cbf", tag="cbf")
        nc.gpsimd.dma_start(out=cbf_t, in_=ap["cbf"])
        cf4_t = persist.tile([128, 14], F32, name="cf4", tag="cf4")
        nc.gpsimd.dma_start(out=cf4_t, in_=ap["cf4"])

        w1t_t = [w1c_t[:, cc * WID:(cc + 1) * WID] for cc in range(CC1)]
        off = 0
        w3t_t = [cbf_t[:, off + oc * COUT:off + (oc + 1) * COUT]
                 for oc in range(CCW)]
        off += CCW * COUT
        gbt_t = [cbf_t[:, off + cc * RANK:off + (cc + 1) * RANK]
                 for cc in range(CC3)]
        off += CC3 * RANK
        i128_t = cbf_t[:, off:off + 128]
        ident_t = cbf_t[0:64, off + 128:off + 128 + 64]
        b1_t = [cf4_t[:, c:c + 1] for c in range(CCW)]
        b2_t = [cf4_t[:, 2 + c:3 + c] for c in range(CCW)]
        s2_t = [cf4_t[:, 4 + c:5 + c] for c in range(CCW)]
        b3_t = [cf4_t[:, 6 + c:7 + c] for c in range(CC3)]
        if variant == "lr":
            ga_t = persist.tile([RANK + 1, COUT], BF16, name="ga", tag="ga")
            nc.gpsimd.dma_start(out=ga_t, in_=ap["ga"])
        else:
            gd_t = []
            for cc in range(CC3):
                t = persist.tile([128, COUT], BF16, name=f"gd_{cc}",
                                 tag=f"gd{cc}")
                nc.gpsimd.dma_start(out=t, in_=ap["gd"][cc])
                gd_t.append(t)

        out2_t = [persist.tile([128, FR], BF16, name=f"out2_{oc}",
                               tag=f"out2{oc}") for oc in range(CCW)]
        # single resb tile so one DMA stores a 512-slice across all 8 oc
        resb = persist.tile([128, CC3 * FR], BF16, name="resb", tag="resb")
        rv = resb.rearrange("p (c f) -> p c f", c=CC3)
        odv = ap["out"].rearrange("p (c f) -> p c f", c=CC3)

        # out1 fp8, padded width: [128, (c2, h4, w18, n64)], pad cols zeroed
        out1q = persist.tile([128, CCW * HLO * WP * N], FP8, name="out1q",
                             tag="out1q")
        o1v = out1q.rearrange("p (c h w n) -> p c h w n",
                              c=CCW, h=HLO, w=WP)
        nc.gpsimd.memset(o1v[:, :, :, 0, :], 0.0)
        nc.gpsimd.memset(o1v[:, :, :, W + 1, :], 0.0)

        lct_pool = ctx.enter_context(tc.tile_pool(name="lctp", bufs=2))
        div_pool = ctx.enter_context(tc.tile_pool(name="divp", bufs=4))
        yb_t = None
        if variant == "lr":
            # moving operand of inhibition stage 2; row RANK stays 1.0.
            yb_t = [persist.tile([RANK + 1, 512], BF16, name=f"yb{i}",
                                 tag=f"yb{i}") for i in range(2)]
            for t in yb_t:
                nc.gpsimd.memset(t, 1.0)

        def conv1_rows(rows):
            # conv1x1 #1 + BN1 + ReLU -> padded fp8 out1 (skip W-pad cols)
            for h in rows:
                for oc in range(CCW):
                    for q in range(2):
                        ps = psum.tile([128, 512], F32, name="ps1", tag="a",
                                       bufs=2)
                        base = h * (W * N) + q * 512
                        for cc in range(CC1):
                            nc.tensor.matmul(
                                ps,
                                w1t_t[cc][:, oc * 128:(oc + 1) * 128],
                                xball[:,
                                      cc * XBF + base:cc * XBF + base + 512],
                                start=(cc == 0), stop=(cc == CC1 - 1))
                        nc.scalar.activation(
                            out=o1v[:, oc, h, 1 + 8 * q:9 + 8 * q, :],
                            in_=ps.rearrange("p (w n) -> p w n", n=N),
                            func=AF.Relu, bias=b1_t[oc], scale=1.0)

        if "conv1" in stages:
            conv1_rows(range(3))   # rows 0-2; row 3 traced after slice 1

        if "lcmm" not in stages:
            for oc in range(CCW):
                nc.gpsimd.memset(out2_t[oc], 0.01)
        lw_shared = None
        if "lcdma" not in stages and "lcmm" in stages:
            lw_shared = persist.tile([128, LPC * 9 * 2 * WID], FP8,
                                     name="lw_shared", tag="lws")
            nc.gpsimd.memset(lw_shared, 0.01)
        if "conv3" not in stages:
            nc.gpsimd.memset(resb, 0.01)

        for ck in range(NCHUNK):
            # -- LC chunk: 4 locations -> out2 cols [ck*256, ck*256+256) --
            if "lcmm" in stages:
                lw = lw_tiles[ck] if "lcdma" in stages else lw_shared
                lwv = lw.rearrange("p (l dk c o) -> p l dk c o",
                                   l=LPC, dk=9, c=2)
                pst_all = psum.tile([128, CCW * LPC * N], BF16, name="pst",
                                    tag="tp", bufs=1)
                pst = [pst_all[:, oc * LPC * N:(oc + 1) * LPC * N]
                       for oc in range(CCW)]
                for lp in range(LPC // 2):
                    # two locations accumulate into one PSUM bank so the
                    # psum->sbuf copy is one big transfer, alternating
                    # between DVE and Act so neither paces the LC loop
                    ps2 = psum.tile([64, 2 * WID], F32, name="ps2",
                                    tag="lc", bufs=2)
                    for half in range(2):
                        li = lp * 2 + half
                        loc = ck * LPC + li
                        hl, j = divmod(loc, W)
                        po = half * WID
                        for dk in range(9):
                            di, dj = divmod(dk, 3)
                            nc.tensor.matmul(
                                ps2[:, po:po + WID],
                                o1v[:, :, hl + di, j + dj, :],
                                lwv[:, li, dk],
                                start=(dk == 0), stop=(dk == 8),
                                perf_mode=DR)
                    tmpb = lct_pool.tile([64, 2 * WID], BF16, name="tmpb",
                                         tag="tmpb")
                    if lp % 2 == 0:
                        nc.vector.tensor_copy(out=tmpb, in_=ps2)
                    else:
                        nc.scalar.activation(out=tmpb, in_=ps2,
                                             func=AF.Copy, scale=1.0)
                    for half in range(2):
                        li = lp * 2 + half
                        for oc in range(CCW):
                            hb = half * WID + oc * 128
                            nc.tensor.transpose(
                                pst[oc][:, li * N:(li + 1) * N],
                                tmpb[:, hb:hb + 128], ident_t)
                for oc in range(CCW):
                    nc.scalar.activation(
                        out=out2_t[oc][:, ck * LPC * N:(ck + 1) * LPC * N],
                        in_=pst[oc], func=AF.Relu, bias=b2_t[oc],
                        scale=s2_t[oc])

            def do_slice(ns, c0, c1, alt_mult):
                # conv3 + BN3 + residual + ReLU + inhibition + divide on
                # out2/resb columns [c0, c1)
                w = c1 - c0
                sl = slice(c0, c1)
                for oc3 in range(CC3 if "conv3" in stages else 0):
                    ps = psum.tile([128, 512], F32, name="ps3", tag="a",
                                   bufs=2)
                    for oc in range(CCW):
                        nc.tensor.matmul(
                            ps[:, :w],
                            w3t_t[oc][:, oc3 * 128:(oc3 + 1) * 128],
                            out2_t[oc][:, sl],
                            start=(oc == 0), stop=False)
                    # residual add rides the PSUM accumulator: ps += I @ x
                    rb = oc3 * XBF + W * N + c0
                    nc.tensor.matmul(ps[:, :w], i128_t,
                                     xball[:, rb:rb + w],
                                     start=False, stop=True)
                    # resb = relu(ps + beta3); alternate Act/DVE so the 8
                    # BN3 ops do not serialize on one engine ahead of yps
                    if oc3 % 2 == 0:
                        nc.scalar.activation(out=rv[:, oc3, sl],
                                             in_=ps[:, :w], func=AF.Relu,
                                             bias=b3_t[oc3], scale=1.0)
                    else:
                        nc.vector.tensor_scalar(
                            out=rv[:, oc3, sl], in0=ps[:, :w],
                            scalar1=b3_t[oc3], scalar2=0.0,
                            op0=ALU.add, op1=ALU.max)
                if "inhib" not in stages:
                    return
                yo = c0 % 512
                if variant == "lr":
                    yps = psum.tile([RANK, 512], F32, name="yps",
                                    tag="lc", bufs=2)
                    for cc in range(CC3):
                        nc.tensor.matmul(yps[:, :w], gbt_t[cc],
                                         rv[:, cc, sl], start=(cc == 0),
                                         stop=(cc == CC3 - 1))
                    yb = yb_t[ns % 2]
                    nc.vector.tensor_copy(out=yb[:RANK, yo:yo + w],
                                          in_=yps[:, :w])
                for oc in range(CC3):
                    ps = psum.tile([128, 512], F32, name="ps4", tag="s4",
                                   bufs=3)
                    if variant == "lr":
                        # lhsT row RANK is ones, yb row RANK is ones:
                        # psum = inh + 1 directly
                        nc.tensor.matmul(
                            ps[:, :w], ga_t[:, oc * 128:(oc + 1) * 128],
                            yb[:, yo:yo + w], start=True, stop=True)
                        den = ps[:, :w]
                    else:
                        for cc in range(CC3):
                            nc.tensor.matmul(
                                ps[:, :w],
                                gd_t[cc][:, oc * 128:(oc + 1) * 128],
                                rv[:, cc, sl],
                                start=(cc == 0), stop=(cc == CC3 - 1))
                        den = div_pool.tile([128, 512], F32, name="den",
                                            tag="den")[:, :w]
                        nc.scalar.add(out=den, in_=ps[:, :w], add=1.0)
                    # rec = 1/(1+inh) on DVE (only engine with tensor
                    # reciprocal); final multiply on Pool (DVE joins at
                    # the tail where it has no LC copies left)
                    rec = div_pool.tile([128, 512], F32, name="rec",
                                        tag="rec")
                    nc.vector.reciprocal_approx_fast(out=rec[:, :w],
                                                     in_=den)
                    feng = (nc.gpsimd, nc.vector)[oc % 2] if alt_mult \
                        else nc.gpsimd
                    feng.tensor_tensor(out=rv[:, oc, sl],
                                       in0=rv[:, oc, sl],
                                       in1=rec[:, :w], op=ALU.mult)

            if ck == 1 and "conv1" in stages:
                conv1_rows([3])
            if ck % 2 == 1 and ck < NCHUNK - 1:
                # full 512-slice ns = ck//2
                ns = ck // 2
                do_slice(ns, ns * 512, ns * 512 + 512, alt_mult=False)
                nc.sync.dma_start(out=odv[:, :, ns * 512:ns * 512 + 512],
                                  in_=rv[:, :, ns * 512:ns * 512 + 512])
            elif ck == NCHUNK - 2:
                # first half of the last slice: only needs chunk 6, so it
                # runs while chunk 7 is still streaming in
                do_slice(3, 1536, 1792, alt_mult=False)
            elif ck == NCHUNK - 1:
                # second half of the last slice: short 256-wide tail chain
                do_slice(3, 1792, 2048, alt_mult=True)
                nc.sync.dma_start(out=odv[:, :4, 1536:2048],
                                  in_=rv[:, :4, 1536:2048])
                nc.sync.dma_start(out=odv[:, 4:, 1536:2048],
                                  in_=rv[:, 4:, 1536:2048])



def _pow2_scale(maxabs, target=120.0):
    return 2.0 ** np.floor(np.log2(target / np.maximum(maxabs, 1e-30)))


def _prep_inputs(x, w1, g1, b1, m1, v1, lc_w, g2, b2, m2, v2,
                 w3, g3, b3, m3, v3, sigmas):
    """Host-side shard + layout prep. Returns (variant, per-core maps)."""
    f4 = np.float32
    x = np.asarray(x, f4)
    inv1 = (g1 / np.sqrt(v1 + EPS)).astype(f4)
    beta1 = (b1 - m1 * inv1).astype(f4)
    inv2 = (g2 / np.sqrt(v2 + EPS)).astype(f4)
    beta2 = (b2 - m2 * inv2).astype(f4)
    inv3 = (g3 / np.sqrt(v3 + EPS)).astype(f4)
    beta3 = (b3 - m3 * inv3).astype(f4)

    w1t = (np.asarray(w1, f4) * inv1[:, None]).T.reshape(CC1, 128, WID)
    w1t = np.ascontiguousarray(w1t).astype(NPBF16)
    w3t = (np.asarray(w3, f4) * inv3[:, None]).T.reshape(CCW, 128, COUT)
    w3t = np.ascontiguousarray(w3t).astype(NPBF16)

    # lc_w: (1,O,C,H,W,9) -> fp8 [h, w, p, (dk, ch, o)] with c = ch*128+p,
    # scaled per out-channel to a power of 2 (undone by BN2's act scale).
    lcw = np.asarray(lc_w[0], f4) * inv2[:, None, None, None, None]
    s2m = _pow2_scale(np.abs(lcw).max(axis=(1, 2, 3, 4)))   # (O,)
    lcw *= s2m[:, None, None, None, None]
    lcw = lcw.transpose(2, 3, 1, 4, 0)             # (H, W, C, 9, O)
    lcw = lcw.reshape(H, W, CCW, 128, 9, WID)      # (h, w, ch, p, dk, o)
    lcw = lcw.transpose(0, 1, 3, 4, 2, 5)          # (h, w, p, dk, ch, o)
    lcw = np.clip(lcw, -240.0, 240.0)
    lcw = np.ascontiguousarray(
        lcw.reshape(H, W, 128, 9 * 2 * WID)).astype(NPFP8)
    s2inv = (1.0 / s2m).astype(f4)

    # x bf16: (C, Hpad, W, N), rows zero-padded at both ends
    xt = np.zeros((CIN, H + 2, W, N), f4)
    xt[:, 1:H + 1] = x.transpose(1, 2, 3, 0)
    xtb = xt.astype(NPBF16)

    # inhibition mixing matrix g on host (fp32), then SVD -> low rank
    idx = np.arange(COUT)
    ci = np.abs(idx + 1.0 - (COUT // 2 + 1.0))
    dist = ci[(idx[None, :] - idx[:, None]) % COUT]          # (O, C)
    sig = np.maximum(np.asarray(sigmas, np.float64), 0.5)
    g = np.exp(-dist.astype(np.float64) ** 2 / (2.0 * sig ** 2)) / sig
    g = g / g.sum(axis=0)                                     # (O, C)
    U, S, Vt = np.linalg.svd(g)
    tail = float(S[RANK] / S[0]) if S.shape[0] > RANK else 0.0
    variant = "lr" if tail < 1e-3 else "dense"
    if variant == "lr":
        A = (U[:, :RANK] * S[:RANK]).astype(f4)               # (O, r)
        B = Vt[:RANK].astype(f4)                              # (r, C)
        ga = np.concatenate([A.T, np.ones((1, COUT), f4)])    # (r+1, O)
        gbt = B.T.reshape(CC3, 128, RANK).astype(f4)          # (cc,p,r)
    else:
        gbt = np.zeros((CC3, 128, RANK), f4)

    # packed bf16 consts: [w3t | gbt | I128 | I64] along the free dim
    eye64 = np.zeros((128, 64), f4)
    eye64[:64, :64] = np.eye(64, dtype=f4)
    cbf = np.concatenate(
        [w3t.transpose(1, 0, 2).reshape(128, CCW * COUT).astype(f4),
         gbt.transpose(1, 0, 2).reshape(128, CC3 * RANK),
         np.eye(128, dtype=f4),
         eye64],
        axis=1).astype(NPBF16)
    w1c = np.ascontiguousarray(
        w1t.transpose(1, 0, 2).reshape(128, CC1 * WID).astype(f4)
    ).astype(NPBF16)
    # packed f32 consts: [b1(2) b2(2) s2(2) b3(8)] as columns
    cf4 = np.concatenate(
        [beta1.reshape(CCW, 128).T, beta2.reshape(CCW, 128).T,
         s2inv.reshape(CCW, 128).T, beta3.reshape(CC3, 128).T],
        axis=1).astype(f4)
    com = {
        "w1c": w1c,
        "cbf": np.ascontiguousarray(cbf),
        "cf4": np.ascontiguousarray(cf4),
    }
    if variant == "lr":
        com["ga"] = np.ascontiguousarray(ga).astype(NPBF16)
    else:
        # device layout [c, o]: gd[cc][p, o] = g[o, cc*128+p]
        com["gd"] = np.ascontiguousarray(
            g.T.astype(f4).reshape(CC3, 128, COUT)).astype(NPBF16)

    in_maps = []
    for r in range(NCORES):
        r0 = r * RPC
        # x: (C, HLO, W, N) -> [128, (cc, h, w, n)]
        xbc = np.ascontiguousarray(
            xtb[:, r0:r0 + HLO].reshape(CC1, 128, XBF).transpose(1, 0, 2)
        ).reshape(128, CC1 * XBF)
        lw = np.ascontiguousarray(lcw[r0:r0 + RPC]).reshape(
            NLOC, 128, 9 * 2 * WID)
        if r == 0 or r == NCORES - 1:
            lw = lw.copy()
            if r == 0:           # row 0 locations: di=0 taps read row -1
                lw[0:W, :, 0:3 * 2 * WID] = 0
            if r == NCORES - 1:  # row 15 locations: di=2 taps read row 16
                lw[W:2 * W, :, 6 * 2 * WID:] = 0
        # group 4 locations per DMA chunk: [8, 128, 4*4608]
        lw = np.ascontiguousarray(
            lw.reshape(NCHUNK, LPC, 128, 9 * 2 * WID).transpose(0, 2, 1, 3)
        ).reshape(NCHUNK, 128, LPC * 9 * 2 * WID)
        in_maps.append(dict(com, xb=xbc, lcw=lw))
    return variant, in_maps


def _assemble(results):
    """results: per-core dicts with 'out' [128, CC3*FR] bf16 -> (N,C,H,W)"""
    full = np.empty((N, COUT, H, W), np.float32)
    for r, res in enumerate(results):
        o = np.asarray(res["out"]).astype(np.float32)
        o = o.reshape(128, CC3, RPC, W, N)
        # (p, cc, hl, j, n) -> (n, c=cc*128+p, h, w)
        o = o.transpose(4, 1, 0, 2, 3).reshape(N, COUT, RPC, W)
        full[:, :, r * RPC:(r + 1) * RPC, :] = o
    return full


_NC_CACHE = {}


def get_nc(ktimes: int = 1, variant: str = "lr", stages=ALL_STAGES):
    key = (ktimes, variant, tuple(stages))
    if key not in _NC_CACHE:
        _NC_CACHE[key] = _build_nc(ktimes, variant, stages)
    return _NC_CACHE[key]


def kernel(**inputs):
    variant, in_maps = _prep_inputs(**inputs)
    nc = get_nc(1, variant)
    res = run_bass_kernel_spmd(nc, in_maps, core_ids=list(range(NCORES)))
    return _assemble(res.results)


if __name__ == "__main__":
    rng = np.random.default_rng(0)
    ins = {
        "x": rng.standard_normal((N, CIN, H, W)).astype(np.float32),
        "w1": (rng.standard_normal((WID, CIN)).astype(np.float32) * 0.05),
        "g1": rng.random(WID).astype(np.float32),
        "b1": rng.standard_normal(WID).astype(np.float32) * 0.05,
        "m1": np.zeros(WID, np.float32),
        "v1": np.ones(WID, np.float32),
        "lc_w": rng.standard_normal((1, WID, WID, H, W, 9)).astype(
            np.float32) * 0.05,
        "g2": rng.random(WID).astype(np.float32),
        "b2": rng.standard_normal(WID).astype(np.float32) * 0.05,
        "m2": np.zeros(WID, np.float32),
        "v2": np.ones(WID, np.float32),
        "w3": rng.standard_normal((COUT, WID)).astype(np.float32) * 0.05,
        "g3": rng.random(COUT).astype(np.float32),
        "b3": rng.standard_normal(COUT).astype(np.float32) * 0.05,
        "m3": np.zeros(COUT, np.float32),
        "v3": np.ones(COUT, np.float32),
        "sigmas": rng.random(COUT).astype(np.float32) + COUT / 8.0,
    }
    out = kernel(**ins)
    print("out", out.shape, out.dtype, float(np.abs(out).max()))
